# revision 44
# baseline (speedup 1.0000x reference)
"""Fused self-attention (FCSelfAttention) Trainium2 Bass kernel.

Problem: X:[4,2048,512] fp32, W_qkv:[512,1536], W_out:[512,512], b_out:[512]
  qkv = X @ W_qkv ; q,k,v -> heads (B,H=8,N=2048,DH=64)
  scores[n,m] = k_n . q_m * DH**-0.5 ; softmax over m (query axis)
  out[n] = sum_e att[n,e] v[e] ; merge heads ; @ W_out + b_out

Sharding (8 cores): batch x head-group. Core c handles batch b=c//2 and
heads 4g..4g+3 where g=c%2 (data parallel over B=4, tensor parallel over
H=8 in halves). Each core computes a partial output projection for its
batch; the host sums the two partials per batch and adds b_out.

Device algorithm (per core), flash-style with scores kept transposed so
the softmax axis lands on the TensorE contraction axis:
  S^T[m,n] = sum_d QT[d,m] KT[d,n]        (m = softmax axis, on partitions)
  P^T = exp(S^T * SCALE)                   (no max subtraction; |S| < 9)
  PV:  lhsT = V_aug[e, 0:65] (col 64 = ones) -> psum[0:64]=out^T, psum[64]=Z

Schedule: heads are processed in pairs whose score matmuls (K=64) run
concurrently on disjoint PE row groups (base partitions 0/64); n is
processed in 512-wide quarters so each PV accumulator is one PSUM bank
(psA 2x[128,1024] score slots + 2 PV accumulators + 2 out-proj slots =
exactly 8 banks).  ACT (exp) is the pacing engine; all other work -
remaining V tiles, pair-1 q/k projections, 1/Z chains (DRAM-bounce
repartition of the Z row, PE transposes for the final quarter), paired
K=64 out-projections (per-head 1/Z applied as a per-partition scalar on
the psum t axis), and output stores - is deferred through pending
queues that drain one op per ec iteration inside later quarters' loops,
filling PE/DVE/DMA bubbles without stalling the exp pipeline.
"""

import sys

import numpy as np

_B, _N, _DIM = 4, 2048, 512
_H, _DH = 8, 64
_SCALE = _DH ** -0.5
_NCORES = 8
_HPC = 4              # heads per core
_HL = _HPC * _DH      # 256 local inner dim
_TC = _N // 128       # 16 token chunks
_KC = _DIM // 128     # 4 contraction chunks for projections

_cache = {}


def _emit(tc, xt, wq, wk, wv, wo, out, mybir):
    nc = tc.nc
    dt = mybir.dt
    f32, bf16 = dt.float32, dt.bfloat16
    Exp = mybir.ActivationFunctionType.Exp
    Alu = mybir.AluOpType

    from contextlib import ExitStack

    with ExitStack() as ctx:
        weights = ctx.enter_context(tc.tile_pool(name="weights", bufs=1))
        xtp = ctx.enter_context(tc.tile_pool(name="xtp", bufs=1))
        qkp = ctx.enter_context(tc.tile_pool(name="qkp", bufs=1))
        vap = ctx.enter_context(tc.tile_pool(name="vap", bufs=1))
        atp = ctx.enter_context(tc.tile_pool(name="atp", bufs=1))
        ptp = ctx.enter_context(tc.tile_pool(name="ptp", bufs=4))
        zp = ctx.enter_context(tc.tile_pool(name="zp", bufs=2))
        zdp = ctx.enter_context(tc.tile_pool(name="zdp", bufs=2, space="DRAM"))
        outp = ctx.enter_context(tc.tile_pool(name="outp", bufs=1))
        psA = ctx.enter_context(tc.tile_pool(name="psA", bufs=2, space="PSUM"))
        psOp = ctx.enter_context(tc.tile_pool(name="psO", bufs=2, space="PSUM"))
        psB = ctx.enter_context(tc.tile_pool(name="psB", bufs=2, space="PSUM"))

        # ---- load inputs (spread across engine DMA queues) ---------------
        dma_engines = [nc.scalar, nc.sync, nc.gpsimd, nc.scalar]
        xt_sb = []
        for kc in range(_KC):
            t = xtp.tile([128, _N], bf16, tag=f"xt{kc}", name=f"xt{kc}")
            dma_engines[kc].dma_start(t, xt[kc * 128:(kc + 1) * 128, :])
            xt_sb.append(t)

        wq_sb, wk_sb, wv_sb = [], [], []
        for name, dram, lst in (("wq", wq, wq_sb), ("wk", wk, wk_sb),
                                ("wv", wv, wv_sb)):
            for kc in range(_KC):
                t = weights.tile([128, _HL], bf16, tag=f"{name}{kc}",
                                 name=f"{name}{kc}")
                nc.gpsimd.dma_start(t, dram[kc * 128:(kc + 1) * 128, :])
                lst.append(t)
        wo_sb = []
        for pair in range(2):
            t = weights.tile([128, _DIM], bf16, tag=f"wo{pair}",
                             name=f"wo{pair}")
            nc.gpsimd.dma_start(t, wo[pair * 128:(pair + 1) * 128, :])
            wo_sb.append(t)
        ones11 = weights.tile([1, 1], f32, tag="ones11", name="ones11")
        nc.vector.memset(ones11, 1.0)

        # Warm the PE HAM clock with dummy matmuls while input DMAs land.
        dummy = xtp.tile([128, 512], bf16, tag="dummy", name="dummy")
        nc.vector.memset(dummy, 0.0)
        for _ in range(3):
            psw = psA.tile([128, 512], f32, tag="mm")
            for _ in range(8):
                nc.tensor.matmul(psw, lhsT=dummy[:, 0:128], rhs=dummy,
                                 start=True, stop=True)

        # ---- qkv projections --------------------------------------------
        # QT/KT: [hd, t] (2 chunks of 128 rows = 2 heads each).  Pair 0's
        # projections and V are emitted up front; pair 1's are deferred
        # into pair 0's attention loop via the pending queue.
        qt_sb = [None, None]
        kt_sb = [None, None]

        def project_qk_piece(name, wsb, lst, hc, tp, pool=None, tag="mm"):
            if lst[hc] is None:
                lst[hc] = qkp.tile([128, _N], bf16, tag=f"{name}{hc}",
                                   name=f"{name}{hc}")
            dst = lst[hc]
            ps = (pool or psA).tile([128, 512], f32, tag=tag)
            for kc in range(_KC):
                nc.tensor.matmul(
                    ps,
                    lhsT=wsb[kc][:, hc * 128:(hc + 1) * 128],
                    rhs=xt_sb[kc][:, tp * 512:(tp + 1) * 512],
                    start=(kc == 0), stop=(kc == _KC - 1),
                )
            nc.vector.tensor_copy(dst[:, tp * 512:(tp + 1) * 512], ps)

        def project_qk(name, wsb, lst, hc):
            for tp in range(_N // 512):
                project_qk_piece(name, wsb, lst, hc, tp)

        # V augmented with a ones column: va[t][:, h, 0:64] = V, [..., 64]=1
        va_sb = []
        for t in range(_TC):
            va_sb.append(vap.tile([128, _HPC, 65], bf16, tag=f"va{t}",
                                  name=f"va{t}"))

        def v_piece(t, pool=None, tag="mm"):
            va = va_sb[t]
            nc.gpsimd.memset(va[:, :, 64:65], 1.0)
            ps = (pool or psA).tile([128, _HL], f32, tag=tag)
            for kc in range(_KC):
                nc.tensor.matmul(
                    ps,
                    lhsT=xt_sb[kc][:, t * 128:(t + 1) * 128],
                    rhs=wv_sb[kc],
                    start=(kc == 0), stop=(kc == _KC - 1),
                )
            nc.vector.tensor_copy(
                va[:, :, 0:64], ps.rearrange("p (h d) -> p h d", h=_HPC))

        # Minimum serial prefix before attention can start: qt0/kt0 piece 0
        # and the first few V tiles; everything else is woven into the
        # attention loop's bubble slots in need-by order (one pop per ec).
        V_UPFRONT = 6
        project_qk("qt", wq_sb, qt_sb, 0)
        project_qk_piece("kt", wk_sb, kt_sb, 0, 0)
        for t in range(V_UPFRONT):
            v_piece(t)

        def mkv(t):
            return lambda: v_piece(t, pool=psB, tag="mo")

        def mkp(name, wsb, lst, hc, tp):
            return lambda: project_qk_piece(name, wsb, lst, hc, tp,
                                            pool=psB, tag="mo")

        deferred_fast = [mkv(t) for t in range(V_UPFRONT, _TC)]
        deferred_fast.extend([mkp("kt", wk_sb, kt_sb, 0, 1),
                              mkp("kt", wk_sb, kt_sb, 0, 2),
                              mkp("kt", wk_sb, kt_sb, 0, 3)])

        # pair 1's projections drain at a slower rate (during quarters 1-3)
        deferred_qk = []
        for tp in range(_N // 512):
            def mkq(tp):
                return lambda: project_qk_piece("qt", wq_sb, qt_sb, 1, tp,
                                                pool=psB, tag="mo")

            def mkk(tp):
                return lambda: project_qk_piece("kt", wk_sb, kt_sb, 1, tp,
                                                pool=psB, tag="mo")

            deferred_qk.append(mkk(tp))
            deferred_qk.append(mkq(tp))

        # ---- attention (paired heads) with pipelined out-projection ------
        acc = []
        for t in range(_TC):
            acc.append(outp.tile([128, _DIM], f32, tag=f"acc{t}",
                                 name=f"acc{t}"))
        at_sb = [None, None]        # per pair, [128, N] (head rows stacked)
        zrec = [None] * _HPC

        def outproj_chunk(pair, t, store):
            # two K=64 matmuls run concurrently on disjoint PE row groups;
            # per-head 1/Z scaling happens in the psum->acc RMW ops.
            h0, h1 = 2 * pair, 2 * pair + 1
            tsl = slice(t * 128, (t + 1) * 128)
            ps0 = psB.tile([128, _DIM], f32, tag="mo")
            ps1 = psB.tile([128, _DIM], f32, tag="mo")
            nc.tensor.matmul(ps0, lhsT=at_sb[pair][0:64, tsl],
                             rhs=wo_sb[pair][0:64, :], start=True, stop=True)
            nc.tensor.matmul(ps1, lhsT=at_sb[pair][64:128, tsl],
                             rhs=wo_sb[pair][64:128, :], start=True, stop=True)
            if h0 == 0:
                nc.vector.tensor_scalar_mul(acc[t], ps0, zrec[h0][:, t:t + 1])
            else:
                nc.vector.scalar_tensor_tensor(
                    out=acc[t], in0=ps0, scalar=zrec[h0][:, t:t + 1],
                    in1=acc[t], op0=Alu.mult, op1=Alu.add,
                )
            nc.vector.scalar_tensor_tensor(
                out=acc[t], in0=ps1, scalar=zrec[h1][:, t:t + 1],
                in1=acc[t], op0=Alu.mult, op1=Alu.add,
            )
            if store:
                (nc.gpsimd if t % 2 else nc.sync).dma_start(
                    out[tsl, :], acc[t])

        NQ = 4                      # n-quarters; po = [65, 512] = 1 bank
        # Out-proj / store work from quarter q is deferred and drained one
        # op per ec-iteration inside quarter q+1, so PE (in-order) never
        # stalls the exp pipeline at quarter boundaries.
        pending = list(deferred_fast)
        pending_slow = list(deferred_qk)
        for pair in range(2):
            h0, h1 = 2 * pair, 2 * pair + 1
            at_sb[pair] = atp.tile([128, _N], bf16, tag=f"at{pair}",
                                   name=f"at{pair}")
            for h in (h0, h1):
                zrec[h] = zp.tile([128, _TC], f32, tag=f"zrec{h}",
                                  name=f"zrec{h}", bufs=1)
            zrow0 = zp.tile([1, _N], f32, tag=f"zrow{h0}", name=f"zrow{h0}",
                            bufs=1)
            zrow1 = zp.tile([1, _N], f32, tag=f"zrow{h1}", name=f"zrow{h1}",
                            bufs=1)
            for q in range(NQ):
                ncol = q * 512
                po0 = psOp.tile([65, 512], f32, tag="po")
                po1 = psOp.tile([65, 512], f32, tag="po")
                for ec in range(_TC):
                    # two heads' score chunks run concurrently on disjoint
                    # PE row groups (base partitions 0 / 64)
                    ps = psA.tile([128, 1024], f32, tag="mm")
                    nc.tensor.matmul(
                        ps[:, 0:512],
                        lhsT=qt_sb[pair][0:64, ec * 128:(ec + 1) * 128],
                        rhs=kt_sb[pair][0:64, ncol:ncol + 512],
                        start=True, stop=True,
                    )
                    nc.tensor.matmul(
                        ps[:, 512:1024],
                        lhsT=qt_sb[pair][64:128, ec * 128:(ec + 1) * 128],
                        rhs=kt_sb[pair][64:128, ncol:ncol + 512],
                        start=True, stop=True,
                    )
                    pt = ptp.tile([128, 1024], bf16, tag="pt")
                    nc.scalar.activation(pt, ps, Exp, scale=_SCALE)
                    nc.tensor.matmul(
                        po0[0:65, :], lhsT=va_sb[ec][:, h0, :],
                        rhs=pt[:, 0:512],
                        start=(ec == 0), stop=(ec == _TC - 1),
                    )
                    nc.tensor.matmul(
                        po1[0:65, :], lhsT=va_sb[ec][:, h1, :],
                        rhs=pt[:, 512:1024],
                        start=(ec == 0), stop=(ec == _TC - 1),
                    )
                    if pending:
                        pending.pop(0)()
                    elif pending_slow and ec % 2 == 0:
                        pending_slow.pop(0)()
                # drain the quarter (stacked at rows; z rows to sbuf)
                qs = slice(ncol, ncol + 512)
                nc.vector.tensor_copy(at_sb[pair][0:64, qs], po0[0:64, :])
                nc.vector.tensor_copy(zrow0[:, qs], po0[64:65, :])
                nc.vector.tensor_copy(at_sb[pair][64:128, qs], po1[0:64, :])
                nc.vector.tensor_copy(zrow1[:, qs], po1[64:65, :])

                # 1/Z columns for this quarter via a DRAM bounce (turns the
                # single-partition z row into [128, 4] so reciprocal is
                # cheap), deferred into the next quarter's ec loop.
                last_q = (pair == 1 and q == NQ - 1)

                def mkz(h, zr_row, q, qs, eng):
                    def zchain():
                        zd = zdp.tile([1, 512], f32, tag=f"zd{h % 2}")
                        eng.dma_start(zd, zr_row[0:1, qs])
                        zcol = zp.tile([128, NQ], f32, tag=f"zcol{h % 2}")
                        eng.dma_start(
                            zcol, zd.rearrange("o (j p) -> (o p) j", p=128))
                        nc.vector.reciprocal(
                            zrec[h][:, q * NQ:(q + 1) * NQ], zcol)
                    return zchain

                def mkz_pe(h, zr_row, q):
                    # tail variant: PE is idle after the last exp, so use
                    # tensor-engine transposes instead of the DMA bounce
                    def zchain():
                        pz = psB.tile([128, NQ], f32, tag="mo")
                        for j in range(NQ):
                            jj = q * NQ + j
                            nc.tensor.transpose(
                                pz[:, j:j + 1],
                                zr_row[0:1, jj * 128:(jj + 1) * 128], ones11)
                        nc.vector.reciprocal(
                            zrec[h][:, q * NQ:(q + 1) * NQ], pz)
                    return zchain

                if last_q:
                    pending.append(mkz_pe(h0, zrow0, q))
                    pending.append(mkz_pe(h1, zrow1, q))
                else:
                    pending.append(mkz(h0, zrow0, q, qs, nc.sync))
                    pending.append(mkz(h1, zrow1, q, qs, nc.gpsimd))
                for j in range(NQ):
                    t = q * NQ + j

                    def mk(pair, t, store):
                        return lambda: outproj_chunk(pair, t, store)

                    pending.append(mk(pair, t, pair == 1))
        while pending:
            pending.pop(0)()


def _build():
    if "/opt/trn_rl_repo" not in sys.path:
        sys.path.insert(0, "/opt/trn_rl_repo")
    from concourse import bacc, mybir
    import concourse.tile as tile

    dt = mybir.dt
    nc = bacc.Bacc("TRN2", target_bir_lowering=False, debug=False,
                   num_devices=_NCORES)
    xt = nc.dram_tensor("xt", [_DIM, _N], dt.bfloat16, kind="ExternalInput").ap()
    wq = nc.dram_tensor("wq", [_DIM, _HL], dt.bfloat16, kind="ExternalInput").ap()
    wk = nc.dram_tensor("wk", [_DIM, _HL], dt.bfloat16, kind="ExternalInput").ap()
    wv = nc.dram_tensor("wv", [_DIM, _HL], dt.bfloat16, kind="ExternalInput").ap()
    wo = nc.dram_tensor("wo", [_HL, _DIM], dt.bfloat16, kind="ExternalInput").ap()
    out = nc.dram_tensor("out", [_N, _DIM], dt.float32, kind="ExternalOutput").ap()

    with tile.TileContext(nc) as tc:
        _emit(tc, xt, wq, wk, wv, wo, out, mybir)
    nc.compile()
    return nc


def _get_nc():
    if "nc" not in _cache:
        _cache["nc"] = _build()
    return _cache["nc"]


def _shard_inputs(X, W_qkv, W_out):
    import ml_dtypes
    bf16 = ml_dtypes.bfloat16
    in_maps = []
    for c in range(_NCORES):
        b, g = c // 2, c % 2
        cols = slice(g * _HL, (g + 1) * _HL)
        in_maps.append({
            "xt": np.ascontiguousarray(X[b].T).astype(bf16),
            "wq": W_qkv[:, 0 * _DIM:][:, cols].astype(bf16),
            "wk": W_qkv[:, 1 * _DIM:][:, cols].astype(bf16),
            "wv": W_qkv[:, 2 * _DIM:][:, cols].astype(bf16),
            "wo": W_out[g * _HL:(g + 1) * _HL, :].astype(bf16),
        })
    return in_maps


def _run(inputs, trace=False):
    if "/opt/trn_rl_repo" not in sys.path:
        sys.path.insert(0, "/opt/trn_rl_repo")
    from concourse.bass_utils import run_bass_kernel_spmd

    X = np.asarray(inputs["X"], dtype=np.float32)
    W_qkv = np.asarray(inputs["W_qkv"], dtype=np.float32)
    W_out = np.asarray(inputs["W_out"], dtype=np.float32)
    b_out = np.asarray(inputs["b_out"], dtype=np.float32)

    nc = _get_nc()
    in_maps = _shard_inputs(X, W_qkv, W_out)
    res = run_bass_kernel_spmd(nc, in_maps, list(range(_NCORES)), trace=trace)

    out = np.empty((_B, _N, _DIM), dtype=np.float32)
    for b in range(_B):
        out[b] = res.results[2 * b]["out"] + res.results[2 * b + 1]["out"] + b_out
    return out, res.exec_time_ns


def kernel(**inputs) -> np.ndarray:
    out, _ = _run(inputs, trace=False)
    return out


# revision 45
# speedup vs baseline: 1.0100x; 1.0100x over previous
"""Fused self-attention (FCSelfAttention) Trainium2 Bass kernel.

Problem: X:[4,2048,512] fp32, W_qkv:[512,1536], W_out:[512,512], b_out:[512]
  qkv = X @ W_qkv ; q,k,v -> heads (B,H=8,N=2048,DH=64)
  scores[n,m] = k_n . q_m * DH**-0.5 ; softmax over m (query axis)
  out[n] = sum_e att[n,e] v[e] ; merge heads ; @ W_out + b_out

Sharding (8 cores): batch x head-group. Core c handles batch b=c//2 and
heads 4g..4g+3 where g=c%2 (data parallel over B=4, tensor parallel over
H=8 in halves). Each core computes a partial output projection for its
batch; the host sums the two partials per batch and adds b_out.

Device algorithm (per core), flash-style with scores kept transposed so
the softmax axis lands on the TensorE contraction axis:
  S^T[m,n] = sum_d QT[d,m] KT[d,n]        (m = softmax axis, on partitions)
  P^T = exp(S^T * SCALE)                   (no max subtraction; |S| < 9)
  PV:  lhsT = V_aug[e, 0:65] (col 64 = ones) -> psum[0:64]=out^T, psum[64]=Z

Schedule: heads are processed in pairs whose score matmuls (K=64) run
concurrently on disjoint PE row groups (base partitions 0/64); n is
processed in 512-wide quarters so each PV accumulator is one PSUM bank
(psA 2x[128,1024] score slots + 2 PV accumulators + 2 out-proj slots =
exactly 8 banks).  ACT (exp) is the pacing engine; all other work -
remaining V tiles, pair-1 q/k projections, 1/Z chains (DRAM-bounce
repartition of the Z row, PE transposes for the final quarter), paired
K=64 out-projections (per-head 1/Z applied as a per-partition scalar on
the psum t axis), and output stores - is deferred through pending
queues that drain one op per ec iteration inside later quarters' loops,
filling PE/DVE/DMA bubbles without stalling the exp pipeline.
"""

import sys

import numpy as np

_B, _N, _DIM = 4, 2048, 512
_H, _DH = 8, 64
_SCALE = _DH ** -0.5
_NCORES = 8
_HPC = 4              # heads per core
_HL = _HPC * _DH      # 256 local inner dim
_TC = _N // 128       # 16 token chunks
_KC = _DIM // 128     # 4 contraction chunks for projections

_cache = {}


def _emit(tc, xt, wq, wk, wv, wo, out, mybir):
    nc = tc.nc
    dt = mybir.dt
    f32, bf16 = dt.float32, dt.bfloat16
    Exp = mybir.ActivationFunctionType.Exp
    Alu = mybir.AluOpType

    from contextlib import ExitStack

    with ExitStack() as ctx:
        weights = ctx.enter_context(tc.tile_pool(name="weights", bufs=1))
        xtp = ctx.enter_context(tc.tile_pool(name="xtp", bufs=1))
        qkp = ctx.enter_context(tc.tile_pool(name="qkp", bufs=1))
        vap = ctx.enter_context(tc.tile_pool(name="vap", bufs=1))
        atp = ctx.enter_context(tc.tile_pool(name="atp", bufs=1))
        ptp = ctx.enter_context(tc.tile_pool(name="ptp", bufs=4))
        zp = ctx.enter_context(tc.tile_pool(name="zp", bufs=2))
        zdp = ctx.enter_context(tc.tile_pool(name="zdp", bufs=2, space="DRAM"))
        outp = ctx.enter_context(tc.tile_pool(name="outp", bufs=1))
        psA = ctx.enter_context(tc.tile_pool(name="psA", bufs=2, space="PSUM"))
        psOp = ctx.enter_context(tc.tile_pool(name="psO", bufs=2, space="PSUM"))
        psB = ctx.enter_context(tc.tile_pool(name="psB", bufs=2, space="PSUM"))

        # ---- load inputs (spread across engine DMA queues) ---------------
        dma_engines = [nc.scalar, nc.sync, nc.gpsimd, nc.scalar]
        xt_sb = []
        for kc in range(_KC):
            t = xtp.tile([128, _N], bf16, tag=f"xt{kc}", name=f"xt{kc}")
            dma_engines[kc].dma_start(t[:, 0:512],
                                      xt[kc * 128:(kc + 1) * 128, 0:512])
            xt_sb.append(t)
        for kc in range(_KC):
            dma_engines[kc].dma_start(xt_sb[kc][:, 512:_N],
                                      xt[kc * 128:(kc + 1) * 128, 512:_N])

        wq_sb, wk_sb, wv_sb = [], [], []
        for name, dram, lst in (("wq", wq, wq_sb), ("wk", wk, wk_sb),
                                ("wv", wv, wv_sb)):
            for kc in range(_KC):
                t = weights.tile([128, _HL], bf16, tag=f"{name}{kc}",
                                 name=f"{name}{kc}")
                nc.gpsimd.dma_start(t, dram[kc * 128:(kc + 1) * 128, :])
                lst.append(t)
        wo_sb = []
        for pair in range(2):
            t = weights.tile([128, _DIM], bf16, tag=f"wo{pair}",
                             name=f"wo{pair}")
            nc.gpsimd.dma_start(t, wo[pair * 128:(pair + 1) * 128, :])
            wo_sb.append(t)
        ones11 = weights.tile([1, 1], f32, tag="ones11", name="ones11")
        nc.vector.memset(ones11, 1.0)

        # Warm the PE HAM clock with dummy matmuls while input DMAs land.
        dummy = xtp.tile([128, 512], bf16, tag="dummy", name="dummy")
        nc.vector.memset(dummy, 0.0)
        psw = psA.tile([128, 512], f32, tag="mm")
        for _ in range(8):
            nc.tensor.matmul(psw, lhsT=dummy[:, 0:128], rhs=dummy,
                             start=True, stop=True)

        # ---- qkv projections --------------------------------------------
        # QT/KT: [hd, t] (2 chunks of 128 rows = 2 heads each).  Pair 0's
        # projections and V are emitted up front; pair 1's are deferred
        # into pair 0's attention loop via the pending queue.
        qt_sb = [None, None]
        kt_sb = [None, None]

        def project_qk_piece(name, wsb, lst, hc, tp, pool=None, tag="mm"):
            if lst[hc] is None:
                lst[hc] = qkp.tile([128, _N], bf16, tag=f"{name}{hc}",
                                   name=f"{name}{hc}")
            dst = lst[hc]
            ps = (pool or psA).tile([128, 512], f32, tag=tag)
            for kc in range(_KC):
                nc.tensor.matmul(
                    ps,
                    lhsT=wsb[kc][:, hc * 128:(hc + 1) * 128],
                    rhs=xt_sb[kc][:, tp * 512:(tp + 1) * 512],
                    start=(kc == 0), stop=(kc == _KC - 1),
                )
            nc.vector.tensor_copy(dst[:, tp * 512:(tp + 1) * 512], ps)

        def project_qk(name, wsb, lst, hc):
            for tp in range(_N // 512):
                project_qk_piece(name, wsb, lst, hc, tp)

        # V augmented with a ones column: va[t][:, h, 0:64] = V, [..., 64]=1
        va_sb = []
        for t in range(_TC):
            va_sb.append(vap.tile([128, _HPC, 65], bf16, tag=f"va{t}",
                                  name=f"va{t}"))

        def v_piece(t, pool=None, tag="mm"):
            va = va_sb[t]
            nc.gpsimd.memset(va[:, :, 64:65], 1.0)
            ps = (pool or psA).tile([128, _HL], f32, tag=tag)
            for kc in range(_KC):
                nc.tensor.matmul(
                    ps,
                    lhsT=xt_sb[kc][:, t * 128:(t + 1) * 128],
                    rhs=wv_sb[kc],
                    start=(kc == 0), stop=(kc == _KC - 1),
                )
            nc.vector.tensor_copy(
                va[:, :, 0:64], ps.rearrange("p (h d) -> p h d", h=_HPC))

        # Minimum serial prefix before attention can start: qt0/kt0 piece 0
        # and the first few V tiles; everything else is woven into the
        # attention loop's bubble slots in need-by order (one pop per ec).
        V_UPFRONT = 6
        project_qk("qt", wq_sb, qt_sb, 0)
        project_qk_piece("kt", wk_sb, kt_sb, 0, 0)
        for t in range(V_UPFRONT):
            v_piece(t)

        def mkv(t):
            return lambda: v_piece(t, pool=psB, tag="mo")

        def mkp(name, wsb, lst, hc, tp):
            return lambda: project_qk_piece(name, wsb, lst, hc, tp,
                                            pool=psB, tag="mo")

        deferred_fast = [mkv(t) for t in range(V_UPFRONT, _TC)]
        deferred_fast.extend([mkp("kt", wk_sb, kt_sb, 0, 1),
                              mkp("kt", wk_sb, kt_sb, 0, 2),
                              mkp("kt", wk_sb, kt_sb, 0, 3)])

        # pair 1's projections drain at a slower rate (during quarters 1-3)
        deferred_qk = []
        for tp in range(_N // 512):
            def mkq(tp):
                return lambda: project_qk_piece("qt", wq_sb, qt_sb, 1, tp,
                                                pool=psB, tag="mo")

            def mkk(tp):
                return lambda: project_qk_piece("kt", wk_sb, kt_sb, 1, tp,
                                                pool=psB, tag="mo")

            deferred_qk.append(mkk(tp))
            deferred_qk.append(mkq(tp))

        # ---- attention (paired heads) with pipelined out-projection ------
        acc = []
        for t in range(_TC):
            acc.append(outp.tile([128, _DIM], f32, tag=f"acc{t}",
                                 name=f"acc{t}"))
        at_sb = [None, None]        # per pair, [128, N] (head rows stacked)
        zrec = [None] * _HPC

        def outproj_chunk(pair, t, store, wide=False):
            # two K=64 matmuls run concurrently on disjoint PE row groups;
            # per-head 1/Z scaling happens in the psum->acc RMW ops.
            # wide=True (tail): borrow the idle PV pool for ps1 so the
            # MM/RMW pipeline is 4 slots deep.
            h0, h1 = 2 * pair, 2 * pair + 1
            tsl = slice(t * 128, (t + 1) * 128)
            ps0 = psB.tile([128, _DIM], f32, tag="mo")
            ps1 = (psOp if wide else psB).tile(
                [128, _DIM], f32, tag="po" if wide else "mo")
            nc.tensor.matmul(ps0, lhsT=at_sb[pair][0:64, tsl],
                             rhs=wo_sb[pair][0:64, :], start=True, stop=True)
            nc.tensor.matmul(ps1, lhsT=at_sb[pair][64:128, tsl],
                             rhs=wo_sb[pair][64:128, :], start=True, stop=True)
            if h0 == 0:
                nc.vector.tensor_scalar_mul(acc[t], ps0, zrec[h0][:, t:t + 1])
            else:
                nc.vector.scalar_tensor_tensor(
                    out=acc[t], in0=ps0, scalar=zrec[h0][:, t:t + 1],
                    in1=acc[t], op0=Alu.mult, op1=Alu.add,
                )
            nc.vector.scalar_tensor_tensor(
                out=acc[t], in0=ps1, scalar=zrec[h1][:, t:t + 1],
                in1=acc[t], op0=Alu.mult, op1=Alu.add,
            )
            if store:
                (nc.gpsimd if t % 2 else nc.sync).dma_start(
                    out[tsl, :], acc[t])

        NQ = 4                      # n-quarters; po = [65, 512] = 1 bank
        # Out-proj / store work from quarter q is deferred and drained one
        # op per ec-iteration inside quarter q+1, so PE (in-order) never
        # stalls the exp pipeline at quarter boundaries.
        pending = list(deferred_fast)
        pending_slow = list(deferred_qk)
        for pair in range(2):
            h0, h1 = 2 * pair, 2 * pair + 1
            at_sb[pair] = atp.tile([128, _N], bf16, tag=f"at{pair}",
                                   name=f"at{pair}")
            for h in (h0, h1):
                zrec[h] = zp.tile([128, _TC], f32, tag=f"zrec{h}",
                                  name=f"zrec{h}", bufs=1)
            zrow0 = zp.tile([1, _N], f32, tag=f"zrow{h0}", name=f"zrow{h0}",
                            bufs=1)
            zrow1 = zp.tile([1, _N], f32, tag=f"zrow{h1}", name=f"zrow{h1}",
                            bufs=1)
            for q in range(NQ):
                ncol = q * 512
                po0 = psOp.tile([65, 512], f32, tag="po")
                po1 = psOp.tile([65, 512], f32, tag="po")
                for ec in range(_TC):
                    # two heads' score chunks run concurrently on disjoint
                    # PE row groups (base partitions 0 / 64)
                    ps = psA.tile([128, 1024], f32, tag="mm")
                    nc.tensor.matmul(
                        ps[:, 0:512],
                        lhsT=qt_sb[pair][0:64, ec * 128:(ec + 1) * 128],
                        rhs=kt_sb[pair][0:64, ncol:ncol + 512],
                        start=True, stop=True,
                    )
                    nc.tensor.matmul(
                        ps[:, 512:1024],
                        lhsT=qt_sb[pair][64:128, ec * 128:(ec + 1) * 128],
                        rhs=kt_sb[pair][64:128, ncol:ncol + 512],
                        start=True, stop=True,
                    )
                    pt = ptp.tile([128, 1024], bf16, tag="pt")
                    nc.scalar.activation(pt, ps, Exp, scale=_SCALE)
                    nc.tensor.matmul(
                        po0[0:65, :], lhsT=va_sb[ec][:, h0, :],
                        rhs=pt[:, 0:512],
                        start=(ec == 0), stop=(ec == _TC - 1),
                    )
                    nc.tensor.matmul(
                        po1[0:65, :], lhsT=va_sb[ec][:, h1, :],
                        rhs=pt[:, 512:1024],
                        start=(ec == 0), stop=(ec == _TC - 1),
                    )
                    if pending:
                        pending.pop(0)()
                    elif pending_slow and ec % 2 == 0:
                        pending_slow.pop(0)()
                # drain the quarter (stacked at rows; z rows to sbuf)
                qs = slice(ncol, ncol + 512)
                nc.vector.tensor_copy(at_sb[pair][0:64, qs], po0[0:64, :])
                nc.vector.tensor_copy(zrow0[:, qs], po0[64:65, :])
                nc.vector.tensor_copy(at_sb[pair][64:128, qs], po1[0:64, :])
                nc.vector.tensor_copy(zrow1[:, qs], po1[64:65, :])

                # 1/Z columns for this quarter via a DRAM bounce (turns the
                # single-partition z row into [128, 4] so reciprocal is
                # cheap), deferred into the next quarter's ec loop.
                last_q = (pair == 1 and q == NQ - 1)

                def mkz(h, zr_row, q, qs, eng):
                    def zchain():
                        zd = zdp.tile([1, 512], f32, tag=f"zd{h % 2}")
                        eng.dma_start(zd, zr_row[0:1, qs])
                        zcol = zp.tile([128, NQ], f32, tag=f"zcol{h % 2}")
                        eng.dma_start(
                            zcol, zd.rearrange("o (j p) -> (o p) j", p=128))
                        nc.vector.reciprocal(
                            zrec[h][:, q * NQ:(q + 1) * NQ], zcol)
                    return zchain

                def mkz_pe(h, zr_row, q):
                    # tail variant: PE is idle after the last exp, so use
                    # tensor-engine transposes instead of the DMA bounce
                    def zchain():
                        pz = psB.tile([128, NQ], f32, tag="mo")
                        for j in range(NQ):
                            jj = q * NQ + j
                            nc.tensor.transpose(
                                pz[:, j:j + 1],
                                zr_row[0:1, jj * 128:(jj + 1) * 128], ones11)
                        nc.vector.reciprocal(
                            zrec[h][:, q * NQ:(q + 1) * NQ], pz)
                    return zchain

                if last_q:
                    pending.append(mkz_pe(h0, zrow0, q))
                    pending.append(mkz_pe(h1, zrow1, q))
                else:
                    pending.append(mkz(h0, zrow0, q, qs, nc.sync))
                    pending.append(mkz(h1, zrow1, q, qs, nc.gpsimd))
                for j in range(NQ):
                    t = q * NQ + j

                    def mk(pair, t, store, wide):
                        return lambda: outproj_chunk(pair, t, store, wide)

                    pending.append(mk(pair, t, pair == 1, last_q))
        while pending:
            pending.pop(0)()


def _build():
    if "/opt/trn_rl_repo" not in sys.path:
        sys.path.insert(0, "/opt/trn_rl_repo")
    from concourse import bacc, mybir
    import concourse.tile as tile

    dt = mybir.dt
    nc = bacc.Bacc("TRN2", target_bir_lowering=False, debug=False,
                   num_devices=_NCORES)
    xt = nc.dram_tensor("xt", [_DIM, _N], dt.bfloat16, kind="ExternalInput").ap()
    wq = nc.dram_tensor("wq", [_DIM, _HL], dt.bfloat16, kind="ExternalInput").ap()
    wk = nc.dram_tensor("wk", [_DIM, _HL], dt.bfloat16, kind="ExternalInput").ap()
    wv = nc.dram_tensor("wv", [_DIM, _HL], dt.bfloat16, kind="ExternalInput").ap()
    wo = nc.dram_tensor("wo", [_HL, _DIM], dt.bfloat16, kind="ExternalInput").ap()
    out = nc.dram_tensor("out", [_N, _DIM], dt.float32, kind="ExternalOutput").ap()

    with tile.TileContext(nc) as tc:
        _emit(tc, xt, wq, wk, wv, wo, out, mybir)
    nc.compile()
    return nc


def _get_nc():
    if "nc" not in _cache:
        _cache["nc"] = _build()
    return _cache["nc"]


def _shard_inputs(X, W_qkv, W_out):
    import ml_dtypes
    bf16 = ml_dtypes.bfloat16
    in_maps = []
    for c in range(_NCORES):
        b, g = c // 2, c % 2
        cols = slice(g * _HL, (g + 1) * _HL)
        in_maps.append({
            "xt": np.ascontiguousarray(X[b].T).astype(bf16),
            "wq": W_qkv[:, 0 * _DIM:][:, cols].astype(bf16),
            "wk": W_qkv[:, 1 * _DIM:][:, cols].astype(bf16),
            "wv": W_qkv[:, 2 * _DIM:][:, cols].astype(bf16),
            "wo": W_out[g * _HL:(g + 1) * _HL, :].astype(bf16),
        })
    return in_maps


def _run(inputs, trace=False):
    if "/opt/trn_rl_repo" not in sys.path:
        sys.path.insert(0, "/opt/trn_rl_repo")
    from concourse.bass_utils import run_bass_kernel_spmd

    X = np.asarray(inputs["X"], dtype=np.float32)
    W_qkv = np.asarray(inputs["W_qkv"], dtype=np.float32)
    W_out = np.asarray(inputs["W_out"], dtype=np.float32)
    b_out = np.asarray(inputs["b_out"], dtype=np.float32)

    nc = _get_nc()
    in_maps = _shard_inputs(X, W_qkv, W_out)
    res = run_bass_kernel_spmd(nc, in_maps, list(range(_NCORES)), trace=trace)

    out = np.empty((_B, _N, _DIM), dtype=np.float32)
    for b in range(_B):
        out[b] = res.results[2 * b]["out"] + res.results[2 * b + 1]["out"] + b_out
    return out, res.exec_time_ns


def kernel(**inputs) -> np.ndarray:
    out, _ = _run(inputs, trace=False)
    return out
